# revision 1
# baseline (speedup 1.0000x reference)
"""Trainium2 Bass kernel for nn_ConditionalAdjunctionModelV3 (Chamfer + segment means).

Contract: kernel(**inputs) takes FULL unsharded inputs and returns the full
output tuple (coherence_scalar (B,1), coherence_spatial (B*PTS,),
affordances_batched (B,A)), distributing work across 8 NeuronCores
(1 sample per core, per the data-parallel sharding hint).

Math: for each sample, with Q = pos points (N,3) and R = reconstructed (M,3),
the PE computes G[n,m] = q.r - |q|^2/2 - |r|^2/2 = -d2[n,m]/2 via a single
K=5 matmul per 128-point chunk (coordinates plus the two norm terms folded
into the contraction).  min_m d2 = -2*max_m G, so the expensive min-reductions
become max-reductions over the PSUM tiles:
  - forward (per-point) min: DVE reduce_max over the free (m) axis
  - backward (per-m) min: DVE running elementwise max into an SBUF accumulator
Final 128-way partition maxes / sqrt / means are tiny and done on host.
"""

import numpy as np

import concourse.bacc as bacc
import concourse.tile as tile
from concourse import mybir
from concourse.bass_utils import run_bass_kernel_spmd

B = 8
PTS = 16384
M = 512
A = 5
K = 5               # contraction: x, y, z, -|q|^2/2 (x1), 1 (x -|r|^2/2)
P = 128             # partitions
NCH = PTS // P      # 128 chunks of 128 points
GRP = 4             # psum banks (matmuls) per DVE op group
NGRP = NCH // GRP   # 32 groups
EPS = 1e-12
NEG_BIG = -3.0e38

F32 = mybir.dt.float32

_cache = {}


def _build_nc():
    nc = bacc.Bacc("TRN2", target_bir_lowering=False, debug=False,
                   enable_asserts=False, num_devices=B)

    qside = nc.dram_tensor("qside", [K, PTS], F32, kind="ExternalInput").ap()
    rside = nc.dram_tensor("rside", [K, M], F32, kind="ExternalInput").ap()
    affr = nc.dram_tensor("affr", [P, A * NCH], F32, kind="ExternalInput").ap()

    fmax_d = nc.dram_tensor("fmax", [P, NCH], F32, kind="ExternalOutput").ap()
    bacc_d = nc.dram_tensor("bacco", [P, GRP * M], F32, kind="ExternalOutput").ap()
    affs_d = nc.dram_tensor("affs", [P, A], F32, kind="ExternalOutput").ap()

    with tile.TileContext(nc) as tc:
        with tc.tile_pool(name="consts", bufs=1) as consts, \
             tc.tile_pool(name="work", bufs=1) as work, \
             tc.tile_pool(name="psum", bufs=2, space="PSUM") as psum:

            qs = consts.tile([K, PTS], F32)
            nc.sync.dma_start(out=qs[:], in_=qside)
            rs = consts.tile([K, M], F32)
            nc.sync.dma_start(out=rs[:], in_=rside)
            af = consts.tile([P, A * NCH], F32)
            nc.sync.dma_start(out=af[:], in_=affr)

            fmax = work.tile([P, NCH], F32)
            bak = work.tile([P, GRP * M], F32)
            nc.vector.memset(bak[:], NEG_BIG)

            for g in range(NGRP):
                pt = psum.tile([P, GRP * M], F32, tag="pt")
                for j in range(GRP):
                    c = g * GRP + j
                    nc.tensor.matmul(
                        pt[:, j * M:(j + 1) * M],
                        qs[:, c * P:(c + 1) * P],
                        rs[:],
                        start=True, stop=True,
                    )
                # forward: per-chunk row-max over m (free axis)
                nc.vector.tensor_reduce(
                    out=fmax[:, g * GRP:(g + 1) * GRP],
                    in_=pt[:].rearrange("p (j m) -> p j m", j=GRP),
                    axis=mybir.AxisListType.X,
                    op=mybir.AluOpType.max,
                )
                # backward: running elementwise max (4 independent stripes)
                nc.vector.tensor_tensor(
                    out=bak[:], in0=bak[:], in1=pt[:],
                    op=mybir.AluOpType.max,
                )

            # affordance partial sums: af[p, a*NCH + c] = aff[c*128+p, a]
            affsum = work.tile([P, A], F32)
            nc.vector.tensor_reduce(
                out=affsum[:],
                in_=af[:].rearrange("p (a c) -> p a c", a=A),
                axis=mybir.AxisListType.X,
                op=mybir.AluOpType.add,
            )

            nc.sync.dma_start(out=fmax_d, in_=fmax[:])
            nc.sync.dma_start(out=bacc_d, in_=bak[:])
            nc.sync.dma_start(out=affs_d, in_=affsum[:])

    nc.compile()
    return nc


def _prep_core_inputs(pos_s, rec_s, aff_s):
    """Build per-core device inputs from one sample's slices (host-side layout)."""
    q = pos_s.astype(np.float64)                       # (PTS, 3)
    qq = (q * q).sum(-1)
    qsd = np.empty((K, PTS), np.float32)
    qsd[0:3] = q.T
    qsd[3] = -0.5 * qq
    qsd[4] = 1.0

    r = rec_s.astype(np.float64)                       # (M, 3)
    rr = (r * r).sum(-1)
    rsd = np.empty((K, M), np.float32)
    rsd[0:3] = r.T
    rsd[3] = 1.0
    rsd[4] = -0.5 * rr

    # affr[p, a*NCH + c] = aff_s[c*128 + p, a]
    affr = np.ascontiguousarray(
        aff_s.reshape(NCH, P, A).transpose(1, 2, 0).reshape(P, A * NCH)
    ).astype(np.float32)
    return {"qside": qsd, "rside": rsd, "affr": affr}


def kernel(pos, reconstructed, affordances, batch):
    pos = np.asarray(pos, np.float32)
    rec = np.asarray(reconstructed, np.float32)
    aff = np.asarray(affordances, np.float32)
    batch = np.asarray(batch)

    if "nc" not in _cache:
        _cache["nc"] = _build_nc()
    nc = _cache["nc"]

    posr = pos.reshape(B, PTS, 3)
    in_maps = [_prep_core_inputs(posr[i], rec[i], aff[i * PTS:(i + 1) * PTS])
               for i in range(B)]

    res = run_bass_kernel_spmd(nc, in_maps, core_ids=list(range(B)))
    outs = res.results

    spatial = np.empty((B, PTS), np.float32)
    fwd_mean = np.empty(B, np.float64)
    bwd_mean = np.empty(B, np.float64)
    aff_out = np.empty((B, A), np.float32)

    for i in range(B):
        om = outs[i]
        fmax = om["fmax"]                   # (P, NCH): rowmax of -d2/2
        per_point_d2 = np.maximum(-2.0 * fmax, EPS)
        sp = np.sqrt(per_point_d2)          # (P, NCH); point n = c*128 + p
        spatial[i] = sp.T.reshape(-1)
        fwd_mean[i] = spatial[i].astype(np.float64).mean()

        bk = om["bacco"]                    # (P, GRP*M)
        bm = bk.reshape(P, GRP, M).max(axis=(0, 1))       # (M,)
        bwd = np.sqrt(np.maximum(-2.0 * bm, EPS))
        bwd_mean[i] = bwd.astype(np.float64).mean()

        aff_out[i] = om["affs"].sum(axis=0) / float(PTS)

    # general sorted-batch fallback for the affordance means (Chamfer part of
    # the reference ignores `batch` entirely, so only this output depends on it)
    expected = np.repeat(np.arange(B), PTS)
    if batch.shape != expected.shape or not np.array_equal(
            batch.astype(np.int64), expected):
        sums = np.zeros((B, A), np.float64)
        np.add.at(sums, batch.astype(np.int64), aff.astype(np.float64))
        counts = np.bincount(batch.astype(np.int64), minlength=B).astype(np.float64)
        aff_out = (sums / np.maximum(counts, 1.0)[:, None]).astype(np.float32)

    coherence_scalar = (0.5 * (fwd_mean + bwd_mean)).astype(np.float32)[:, None]
    coherence_spatial = spatial.reshape(-1)
    return coherence_scalar, coherence_spatial, aff_out


# revision 3
# speedup vs baseline: 1017.9450x; 1017.9450x over previous
"""Trainium2 Bass kernel for nn_ConditionalAdjunctionModelV3 (Chamfer + segment means).

Contract: kernel(**inputs) takes FULL unsharded inputs and returns the full
output tuple (coherence_scalar (B,1), coherence_spatial (B*PTS,),
affordances_batched (B,A)), distributing work across 8 NeuronCores
(1 sample per core, per the data-parallel sharding hint).

Math: for each sample, with Q = pos points (N,3) and R = reconstructed (M,3),
the PE computes G[n,m] = q.r - |q|^2/2 - |r|^2/2 = -d2[n,m]/2 via a single
K=5 matmul per 128-point chunk (coordinates plus the two norm terms folded
into the contraction).  min_m d2 = -2*max_m G, so the expensive min-reductions
become max-reductions over the PSUM tiles:
  - forward (per-point) min: DVE reduce_max over the free (m) axis
  - backward (per-m) min: DVE running elementwise max into an SBUF accumulator
Final 128-way partition maxes / sqrt / means are tiny and done on host.
"""

import numpy as np

import concourse.bacc as bacc
import concourse.tile as tile
from concourse import mybir
from concourse.bass_utils import run_bass_kernel_spmd

B = 8
PTS = 16384
M = 512
A = 5
K = 5               # contraction: x, y, z, -|q|^2/2 (x1), 1 (x -|r|^2/2)
P = 128             # partitions
NCH = PTS // P      # 128 chunks of 128 points
GRP = 4             # psum banks (matmuls) per DVE op group
NGRP = NCH // GRP   # 32 groups
EPS = 1e-12
NEG_BIG = -3.0e38

F32 = mybir.dt.float32

_cache = {}


def _build_nc(reps=1):
    from concourse import bass_isa
    nc = bacc.Bacc("TRN2", target_bir_lowering=False, debug=False,
                   enable_asserts=False, num_devices=B)

    qside = nc.dram_tensor("qside", [K, PTS], F32, kind="ExternalInput").ap()
    rside = nc.dram_tensor("rside", [K, M], F32, kind="ExternalInput").ap()
    affr = nc.dram_tensor("affr", [P, A * NCH], F32, kind="ExternalInput").ap()

    fmax_d = nc.dram_tensor("fmax", [P, NCH], F32, kind="ExternalOutput").ap()
    bwd_d = nc.dram_tensor("bwd", [1, M], F32, kind="ExternalOutput").ap()
    affs_d = nc.dram_tensor("affs", [P, A], F32, kind="ExternalOutput").ap()

    with tile.TileContext(nc) as tc:
        with tc.tile_pool(name="consts", bufs=1) as consts, \
             tc.tile_pool(name="work", bufs=1) as work, \
             tc.tile_pool(name="psum", bufs=2, space="PSUM") as psum:

            qs = consts.tile([K, PTS], F32)
            nc.sync.dma_start(out=qs[:], in_=qside)
            rs = consts.tile([K, M], F32)
            nc.sync.dma_start(out=rs[:], in_=rside)
            af = consts.tile([P, A * NCH], F32)
            nc.sync.dma_start(out=af[:], in_=affr)

            for _rep in range(reps):
                fmax = work.tile([P, NCH], F32, tag="fmax")
                bak = work.tile([P, GRP * M], F32, tag="bak")
                nc.gpsimd.memset(bak[:], NEG_BIG)

                for g in range(NGRP):
                    pt = psum.tile([P, GRP * M], F32, tag="pt")
                    for j in range(GRP):
                        c = g * GRP + j
                        nc.tensor.matmul(
                            pt[:, j * M:(j + 1) * M],
                            qs[:, c * P:(c + 1) * P],
                            rs[:],
                            start=True, stop=True,
                        )
                    # forward: per-chunk row-max over m (free axis)
                    nc.vector.tensor_reduce(
                        out=fmax[:, g * GRP:(g + 1) * GRP],
                        in_=pt[:].rearrange("p (j m) -> p j m", j=GRP),
                        axis=mybir.AxisListType.X,
                        op=mybir.AluOpType.max,
                    )
                    # backward: running elementwise max (4 independent stripes)
                    nc.vector.tensor_tensor(
                        out=bak[:], in0=bak[:], in1=pt[:],
                        op=mybir.AluOpType.max,
                    )

                # merge the 4 stripes -> (P, M), then 128-way partition max
                bmax = work.tile([P, M], F32, tag="bmax")
                nc.vector.tensor_reduce(
                    out=bmax[:],
                    in_=bak[:].rearrange("p (j m) -> p m j", j=GRP),
                    axis=mybir.AxisListType.X,
                    op=mybir.AluOpType.max,
                )
                bred = work.tile([P, M], F32, tag="bred")
                nc.gpsimd.partition_all_reduce(
                    bred[:], bmax[:], 128, bass_isa.ReduceOp.max)

                # affordance partial sums: af[p, a*NCH + c] = aff[c*128+p, a]
                affsum = work.tile([P, A], F32, tag="affsum")
                nc.vector.tensor_reduce(
                    out=affsum[:],
                    in_=af[:].rearrange("p (a c) -> p a c", a=A),
                    axis=mybir.AxisListType.X,
                    op=mybir.AluOpType.add,
                )

            nc.sync.dma_start(out=fmax_d, in_=fmax[:])
            nc.sync.dma_start(out=bwd_d, in_=bred[0:1, :])
            nc.sync.dma_start(out=affs_d, in_=affsum[:])

    nc.compile()
    return nc


def _prep_core_inputs(pos_s, rec_s, aff_s):
    """Build per-core device inputs from one sample's slices (host-side layout)."""
    q = pos_s.astype(np.float64)                       # (PTS, 3)
    qq = (q * q).sum(-1)
    qsd = np.empty((K, PTS), np.float32)
    qsd[0:3] = q.T
    qsd[3] = -0.5 * qq
    qsd[4] = 1.0

    r = rec_s.astype(np.float64)                       # (M, 3)
    rr = (r * r).sum(-1)
    rsd = np.empty((K, M), np.float32)
    rsd[0:3] = r.T
    rsd[3] = 1.0
    rsd[4] = -0.5 * rr

    # affr[p, a*NCH + c] = aff_s[c*128 + p, a]
    affr = np.ascontiguousarray(
        aff_s.reshape(NCH, P, A).transpose(1, 2, 0).reshape(P, A * NCH)
    ).astype(np.float32)
    return {"qside": qsd, "rside": rsd, "affr": affr}


def kernel(pos, reconstructed, affordances, batch):
    pos = np.asarray(pos, np.float32)
    rec = np.asarray(reconstructed, np.float32)
    aff = np.asarray(affordances, np.float32)
    batch = np.asarray(batch)

    if "nc" not in _cache:
        _cache["nc"] = _build_nc()
    nc = _cache["nc"]

    posr = pos.reshape(B, PTS, 3)
    in_maps = [_prep_core_inputs(posr[i], rec[i], aff[i * PTS:(i + 1) * PTS])
               for i in range(B)]

    res = run_bass_kernel_spmd(nc, in_maps, core_ids=list(range(B)))
    outs = res.results

    spatial = np.empty((B, PTS), np.float32)
    fwd_mean = np.empty(B, np.float64)
    bwd_mean = np.empty(B, np.float64)
    aff_out = np.empty((B, A), np.float32)

    for i in range(B):
        om = outs[i]
        fmax = om["fmax"]                   # (P, NCH): rowmax of -d2/2
        per_point_d2 = np.maximum(-2.0 * fmax, EPS)
        sp = np.sqrt(per_point_d2)          # (P, NCH); point n = c*128 + p
        spatial[i] = sp.T.reshape(-1)
        fwd_mean[i] = spatial[i].astype(np.float64).mean()

        bm = om["bwd"][0]                   # (M,): max of -d2/2 over all n
        bwd = np.sqrt(np.maximum(-2.0 * bm, EPS))
        bwd_mean[i] = bwd.astype(np.float64).mean()

        aff_out[i] = om["affs"].sum(axis=0) / float(PTS)

    # general sorted-batch fallback for the affordance means (Chamfer part of
    # the reference ignores `batch` entirely, so only this output depends on it)
    expected = np.repeat(np.arange(B), PTS)
    if batch.shape != expected.shape or not np.array_equal(
            batch.astype(np.int64), expected):
        sums = np.zeros((B, A), np.float64)
        np.add.at(sums, batch.astype(np.int64), aff.astype(np.float64))
        counts = np.bincount(batch.astype(np.int64), minlength=B).astype(np.float64)
        aff_out = (sums / np.maximum(counts, 1.0)[:, None]).astype(np.float32)

    coherence_scalar = (0.5 * (fwd_mean + bwd_mean)).astype(np.float32)[:, None]
    coherence_spatial = spatial.reshape(-1)
    return coherence_scalar, coherence_spatial, aff_out


# revision 9
# speedup vs baseline: 2042.4290x; 2.0064x over previous
"""Trainium2 Bass kernel for nn_ConditionalAdjunctionModelV3 (Chamfer + segment means).

Contract: kernel(**inputs) takes FULL unsharded inputs and returns the full
output tuple (coherence_scalar (B,1), coherence_spatial (B*PTS,),
affordances_batched (B,A)), distributing work across 8 NeuronCores
(1 sample per core, per the data-parallel sharding hint).

Math: for each sample, with Q = pos points (N,3) and R = reconstructed (M,3),
the PE computes G[n,m] = q.r - |q|^2/2 - |r|^2/2 = -d2[n,m]/2 via a single
K=5 matmul per 128-point chunk (coordinates plus the two norm terms folded
into the contraction).  min_m d2 = -2*max_m G, so the expensive min-reductions
become max-reductions over the PSUM tiles:
  - forward (per-point) min: DVE reduce_max over the free (m) axis
  - backward (per-m) min: DVE running elementwise max into an SBUF accumulator
Final 128-way partition maxes / sqrt / means are tiny and done on host.
"""

import numpy as np

import concourse.bacc as bacc
import concourse.tile as tile
from concourse import mybir
from concourse.bass_utils import run_bass_kernel_spmd

B = 8
PTS = 16384
M = 512
A = 5
K = 24              # bf16x3 split contraction (see _prep_core_inputs)
P = 128             # partitions
NCH = PTS // P      # 128 chunks of 128 points
GRP = 4             # psum banks (matmuls) per DVE op group
NGRP = NCH // GRP   # 32 groups
EPS = 1e-12
NEG_BIG = -3.0e38

F32 = mybir.dt.float32
BF16 = mybir.dt.bfloat16

_cache = {}


def _build_nc(reps=1, skip_tt=False, mm_frac=1.0):
    from concourse import bass_isa
    nc = bacc.Bacc("TRN2", target_bir_lowering=False, debug=False,
                   enable_asserts=False, num_devices=B)

    qside = nc.dram_tensor("qside", [K, PTS], BF16, kind="ExternalInput").ap()
    rside = nc.dram_tensor("rside", [K, M], BF16, kind="ExternalInput").ap()
    affr = nc.dram_tensor("affr", [P, A * NCH], F32, kind="ExternalInput").ap()

    fmax_d = nc.dram_tensor("fmax", [P, NCH], F32, kind="ExternalOutput").ap()
    bwd_d = nc.dram_tensor("bwd", [1, M], F32, kind="ExternalOutput").ap()
    affs_d = nc.dram_tensor("affs", [P, A], F32, kind="ExternalOutput").ap()

    with tile.TileContext(nc) as tc:
        with tc.tile_pool(name="consts", bufs=1) as consts, \
             tc.tile_pool(name="work", bufs=1) as work, \
             tc.tile_pool(name="psum", bufs=2, space="PSUM") as psum:

            qs = consts.tile([K, PTS], BF16)
            nc.sync.dma_start(out=qs[:], in_=qside)
            rs = consts.tile([K, M], BF16)
            nc.sync.dma_start(out=rs[:], in_=rside)
            af = consts.tile([P, A * NCH], F32)
            nc.sync.dma_start(out=af[:], in_=affr)

            for _rep in range(reps):
                fmax = work.tile([P, NCH], F32, tag="fmax")
                bak = work.tile([P, GRP * M], F32, tag="bak")
                nc.gpsimd.memset(bak[:], NEG_BIG)

                for g in range(NGRP):
                    pt = psum.tile([P, GRP * M], F32, tag="pt")
                    for j in range(GRP):
                        c = g * GRP + j
                        mcols = int(M * mm_frac)
                        if mcols > 0:
                            nc.tensor.matmul(
                                pt[:, j * M:j * M + mcols],
                                qs[:, c * P:(c + 1) * P],
                                rs[:, :mcols],
                                start=True, stop=True,
                            )
                    # forward: per-chunk row-max over m (free axis)
                    nc.vector.tensor_reduce(
                        out=fmax[:, g * GRP:(g + 1) * GRP],
                        in_=pt[:].rearrange("p (j m) -> p j m", j=GRP),
                        axis=mybir.AxisListType.X,
                        op=mybir.AluOpType.max,
                    )
                    # backward: running elementwise max (4 independent stripes)
                    if not skip_tt:
                        nc.vector.tensor_tensor(
                            out=bak[:], in0=bak[:], in1=pt[:],
                            op=mybir.AluOpType.max,
                        )

                # merge the 4 stripes -> (P, M), then 128-way partition max
                bmax = work.tile([P, M], F32, tag="bmax")
                nc.vector.tensor_reduce(
                    out=bmax[:],
                    in_=bak[:].rearrange("p (j m) -> p m j", j=GRP),
                    axis=mybir.AxisListType.X,
                    op=mybir.AluOpType.max,
                )
                bred = work.tile([P, M], F32, tag="bred")
                nc.gpsimd.partition_all_reduce(
                    bred[:], bmax[:], 128, bass_isa.ReduceOp.max)

                # affordance partial sums: af[p, a*NCH + c] = aff[c*128+p, a]
                affsum = work.tile([P, A], F32, tag="affsum")
                nc.vector.tensor_reduce(
                    out=affsum[:],
                    in_=af[:].rearrange("p (a c) -> p a c", a=A),
                    axis=mybir.AxisListType.X,
                    op=mybir.AluOpType.add,
                )

            nc.sync.dma_start(out=fmax_d, in_=fmax[:])
            nc.sync.dma_start(out=bwd_d, in_=bred[0:1, :])
            nc.sync.dma_start(out=affs_d, in_=affsum[:])

    nc.compile()
    return nc


def _split3(v):
    """Split fp64 array into 3 bf16 parts: v ~= h + m + l (residual ~2^-27 |v|)."""
    import ml_dtypes
    bf = ml_dtypes.bfloat16
    h = v.astype(bf)
    r1 = v - h.astype(np.float64)
    m = r1.astype(bf)
    r2 = r1 - m.astype(np.float64)
    l = r2.astype(bf)
    return h, m, l


def _prep_core_inputs(pos_s, rec_s, aff_s):
    """Build per-core device inputs from one sample's slices (host-side layout).

    PSUM[n,m] = sum_k qside[k,n]*rside[k,m] = q.r - |q|^2/2 - |r|^2/2 = -d2/2
    computed with bf16x3 splits: per coordinate the (h,m,l) x (h,m,l) cross
    terms down to 2^-27 relative; norm terms split the same way against ones.
    """
    import ml_dtypes
    bf = ml_dtypes.bfloat16
    q = pos_s.astype(np.float64)                       # (PTS, 3)
    qq = (q * q).sum(-1)
    r = rec_s.astype(np.float64)                       # (M, 3)
    rr = (r * r).sum(-1)

    qsd = np.empty((K, PTS), bf)
    rsd = np.empty((K, M), bf)
    for d in range(3):
        qh, qm, ql = _split3(q[:, d])
        rh, rm, rl = _split3(r[:, d])
        base = d * 6
        # pairs: (qh,rh) (qh,rm) (qm,rh) (qh,rl) (ql,rh) (qm,rm)
        qsd[base + 0], rsd[base + 0] = qh, rh
        qsd[base + 1], rsd[base + 1] = qh, rm
        qsd[base + 2], rsd[base + 2] = qm, rh
        qsd[base + 3], rsd[base + 3] = qh, rl
        qsd[base + 4], rsd[base + 4] = ql, rh
        qsd[base + 5], rsd[base + 5] = qm, rm
    nqh, nqm, nql = _split3(-0.5 * qq)
    qsd[18], qsd[19], qsd[20] = nqh, nqm, nql
    rsd[18] = rsd[19] = rsd[20] = np.asarray(1.0, bf)
    qsd[21] = qsd[22] = qsd[23] = np.asarray(1.0, bf)
    nrh, nrm, nrl = _split3(-0.5 * rr)
    rsd[21], rsd[22], rsd[23] = nrh, nrm, nrl

    # affr[p, a*NCH + c] = aff_s[c*128 + p, a]
    affr = np.ascontiguousarray(
        aff_s.reshape(NCH, P, A).transpose(1, 2, 0).reshape(P, A * NCH)
    ).astype(np.float32)
    return {"qside": qsd, "rside": rsd, "affr": affr}


def kernel(pos, reconstructed, affordances, batch):
    pos = np.asarray(pos, np.float32)
    rec = np.asarray(reconstructed, np.float32)
    aff = np.asarray(affordances, np.float32)
    batch = np.asarray(batch)

    if "nc" not in _cache:
        _cache["nc"] = _build_nc()
    nc = _cache["nc"]

    posr = pos.reshape(B, PTS, 3)
    in_maps = [_prep_core_inputs(posr[i], rec[i], aff[i * PTS:(i + 1) * PTS])
               for i in range(B)]

    res = run_bass_kernel_spmd(nc, in_maps, core_ids=list(range(B)))
    outs = res.results

    spatial = np.empty((B, PTS), np.float32)
    fwd_mean = np.empty(B, np.float64)
    bwd_mean = np.empty(B, np.float64)
    aff_out = np.empty((B, A), np.float32)

    for i in range(B):
        om = outs[i]
        fmax = om["fmax"]                   # (P, NCH): rowmax of -d2/2
        per_point_d2 = np.maximum(-2.0 * fmax, EPS)
        sp = np.sqrt(per_point_d2)          # (P, NCH); point n = c*128 + p
        spatial[i] = sp.T.reshape(-1)
        fwd_mean[i] = spatial[i].astype(np.float64).mean()

        bm = om["bwd"][0]                   # (M,): max of -d2/2 over all n
        bwd = np.sqrt(np.maximum(-2.0 * bm, EPS))
        bwd_mean[i] = bwd.astype(np.float64).mean()

        aff_out[i] = om["affs"].sum(axis=0) / float(PTS)

    # general sorted-batch fallback for the affordance means (Chamfer part of
    # the reference ignores `batch` entirely, so only this output depends on it)
    expected = np.repeat(np.arange(B), PTS)
    if batch.shape != expected.shape or not np.array_equal(
            batch.astype(np.int64), expected):
        sums = np.zeros((B, A), np.float64)
        np.add.at(sums, batch.astype(np.int64), aff.astype(np.float64))
        counts = np.bincount(batch.astype(np.int64), minlength=B).astype(np.float64)
        aff_out = (sums / np.maximum(counts, 1.0)[:, None]).astype(np.float32)

    coherence_scalar = (0.5 * (fwd_mean + bwd_mean)).astype(np.float32)[:, None]
    coherence_spatial = spatial.reshape(-1)
    return coherence_scalar, coherence_spatial, aff_out
